# revision 23
# baseline (speedup 1.0000x reference)
"""2-layer GAT (PyG GATConv semantics) on 8 Trainium2 NeuronCores via Bass.

Strategy (dst-partitioned message passing):
  - Nodes are dealt to 8 cores (degree-balanced round-robin), 12500/core,
    padded to 12544 = 98 tiles of 128 slots.
  - Node phase (per core): h_ext = x_c @ [W1 | W1@Asrc | W1@Adst] on the PE
    (bf16), giving per-node features h plus attention scalars e_src/e_dst.
    Rows packed as gather-friendly 768B records, AllGather -> global table.
  - Edge phase (per core, edges partitioned by dst): per 128-edge chunk,
    dma_gather source rows (h|e_src) by src index, gather e_dst rows by dst,
    e = leaky_relu(e_src+e_dst), exp on ACT, msg = exp * h on DVE, then
    scatter-add into per-dst-tile PSUM via one-hot matmul
    (psum[slot,:] += OneHot^T @ [msg | exp]).  Softmax division happens once
    per dst tile in the epilogue (max-subtraction is skipped: e is small, so
    exp never overflows and the result is mathematically identical).
  - Layer-2 node compute (hh = h1 @ [W2|W2@a2s|W2@a2d]) is fused into the
    layer-1 epilogue via PE transposes; second AllGather, second edge phase,
    log_softmax epilogue.

SPMD constraint: all 8 cores execute ONE instruction stream, so the chunk
schedule (chunks per (dst-tile, src-bucket)) is the max over cores; cores pad
with dummy edges (one-hot column 255 -> contributes nothing).  src indices are
int16 (dma_gather requirement) relative to one of four 32768-row buckets of
the global table; per-tile edges are grouped by bucket (snake order so runs
merge across tile boundaries).
"""

import sys

sys.path.insert(0, "/opt/trn_rl_repo")

import os

import numpy as np
import ml_dtypes

_STAGE = int(os.environ.get("GAT_STAGE", "4"))  # 1=node,2=+edge1,3=+ag2,4=full

from concourse import bass, bacc, mybir, tile
from concourse.bass_utils import run_bass_kernel_spmd

# problem constants (hardcoded per task spec)
N = 100000
E = 1000000
NFEAT = 512
NHID = 32
HEADS = 8
NCLASS = 40
SLOPE = 0.2

P = 128
CORES = 8
WIN = 8           # chunks per window; dma_gather num_idxs must stay <= 1024
                  # (2048-idx gathers crash the device: SWDGE ring limit)
BUCKET = 32768    # int16 index range per gather bucket

F1 = HEADS * NHID            # 256
ROW1 = 384                   # bf16 elems per L1 gather row: h(256) + esrc-as-f32(16) + pad -> 768B
ROW2 = 128                   # bf16 elems per L2 gather row: hh(40) + esrc2-as-f32(2) + pad -> 256B
EDROW = 64                   # f32 elems per e_dst gather row (256B)

DT = mybir.dt
BF16 = DT.bfloat16
F32 = DT.float32
I16 = DT.int16


# ----------------------------------------------------------------------------
# host-side schedule construction
# ----------------------------------------------------------------------------

def _make_schedule(src, dst, n, nloc, ntile, bucket_rows, cores=CORES, win=WIN):
    """Build the shared chunk schedule and per-core index arrays."""
    nlocp = ntile * P

    # degree-balanced node -> (core, slot) assignment
    deg = np.bincount(dst, minlength=n)
    order = np.argsort(-deg, kind="stable")
    core_of = np.empty(n, np.int32)
    slot_of = np.empty(n, np.int32)
    for c in range(cores):
        idxs = order[c::cores]
        core_of[idxs] = c
        slot_of[idxs] = np.arange(len(idxs), dtype=np.int32)

    ecore = core_of[dst]
    eslot = slot_of[dst]
    etile = eslot // P
    gsrc = core_of[src].astype(np.int64) * nlocp + slot_of[src]
    ebucket = (gsrc // bucket_rows).astype(np.int32)
    nbuck = int(ebucket.max()) + 1 if len(ebucket) else 1
    nbuck = max(nbuck, (cores * nlocp + bucket_rows - 1) // bucket_rows)

    # shared chunk counts per (tile, bucket): max over cores
    counts = np.zeros((cores, ntile, nbuck), np.int64)
    flat = (ecore.astype(np.int64) * ntile + etile) * nbuck + ebucket
    cnt = np.bincount(flat, minlength=cores * ntile * nbuck)
    counts = cnt.reshape(cores, ntile, nbuck)
    K = -(-counts.max(axis=0) // P)  # [ntile, nbuck] chunks

    # chunk list in (tile, snake-bucket) order
    chunk_tile, chunk_bucket = [], []
    tile_first = np.zeros(ntile, np.int64)
    tile_last = np.zeros(ntile, np.int64)
    for t in range(ntile):
        border = range(nbuck) if t % 2 == 0 else range(nbuck - 1, -1, -1)
        tile_first[t] = len(chunk_tile)
        for b in border:
            for _ in range(int(K[t, b])):
                chunk_tile.append(t)
                chunk_bucket.append(b)
        tile_last[t] = len(chunk_tile) - 1
        assert tile_last[t] >= tile_first[t], "tile with zero chunks"
    C = len(chunk_tile)
    chunk_tile = np.array(chunk_tile, np.int64)
    chunk_bucket = np.array(chunk_bucket, np.int64)

    # gathers: maximal runs of constant bucket, split at window grid
    gathers = []  # (c0, c1, bucket, srcidx_col_offset)
    off = 0
    c0 = 0
    for c in range(1, C + 1):
        if c == C or chunk_bucket[c] != chunk_bucket[c0] or c % win == 0:
            nidx = (c - c0) * P
            gathers.append((c0, c, int(chunk_bucket[c0]), off))
            off += nidx // 16
            c0 = c
    TS = off
    windows = [(w, min(w + win, C)) for w in range(0, C, win)]
    dwoff = []  # dstidx col offset per window
    off = 0
    for (w0, w1) in windows:
        dwoff.append(off)
        off += (w1 - w0) * P // 16
    TD = off

    # per-core edge ordering and padded per-chunk arrays
    percore = []
    src_off_all = (gsrc - ebucket.astype(np.int64) * bucket_rows).astype(np.int32)
    for c in range(cores):
        m = np.where(ecore == c)[0]
        # sort by (tile, snake-bucket-pos, src) for gather locality
        sb = np.where(etile[m] % 2 == 0, ebucket[m], nbuck - 1 - ebucket[m])
        o = np.lexsort((gsrc[m], sb, etile[m]))
        m = m[o]
        # group boundaries per (tile, bucket) cell -> place edges into chunks
        src16 = np.zeros((C, P), np.int16)
        dloc16 = np.zeros((C, P), np.int16)
        dlocal = np.full((C, P), 255, np.float32)
        # cell start offsets within the chunk schedule
        cell_chunk0 = {}
        cc = 0
        for t in range(ntile):
            border = range(nbuck) if t % 2 == 0 else range(nbuck - 1, -1, -1)
            for b in border:
                cell_chunk0[(t, b)] = cc
                cc += int(K[t, b])
        et, eb = etile[m], ebucket[m]
        cell_ids = et.astype(np.int64) * nbuck + eb
        # position within cell
        _, inv, cnts = np.unique(cell_ids, return_inverse=True, return_counts=True)
        pos = np.zeros(len(m), np.int64)
        sort_by_cell = np.argsort(inv, kind="stable")  # stable keeps src order
        pos[sort_by_cell] = np.arange(len(m)) - np.repeat(
            np.cumsum(cnts) - cnts, cnts
        )
        base = np.array([cell_chunk0[(int(t_), int(b_))] for t_, b_ in zip(et, eb)],
                        np.int64)
        chunk_of = base + pos // P
        lane_of = pos % P
        src16[chunk_of, lane_of] = src_off_all[m].astype(np.int16)
        dloc16[chunk_of, lane_of] = eslot[m].astype(np.int16)
        dlocal[chunk_of, lane_of] = (eslot[m] % P).astype(np.float32)
        # dummy lanes: copy a valid src offset (repeat pattern); default 0 is
        # always a valid row of every bucket slice except possibly short last
        # bucket -- row 0 of any bucket slice is valid since slices are
        # non-empty by construction.  dloc16 default 0 valid.  dlocal 255.

        # wrap src idx per gather, dst idx per window
        scol = np.zeros((16, TS), np.int16)
        for (g0, g1, b, soff) in gathers:
            seg = src16[g0:g1].reshape(-1)
            scol[:, soff:soff + len(seg) // 16] = seg.reshape(-1, 16).T
        dcol = np.zeros((16, TD), np.int16)
        for (w0, w1), woff in zip(windows, dwoff):
            seg = dloc16[w0:w1].reshape(-1)
            dcol[:, woff:woff + len(seg) // 16] = seg.reshape(-1, 16).T
        percore.append(dict(
            srcidx=np.tile(scol, (8, 1)),
            dstidx=np.tile(dcol, (8, 1)),
            dstlocal=dlocal.T.astype(ml_dtypes.bfloat16),  # [P, C]
        ))

    meta = dict(
        C=C, TS=TS, TD=TD, ntile=ntile, nlocp=nlocp, nbuck=nbuck,
        bucket_rows=bucket_rows, windows=windows, dwoff=dwoff,
        gathers=gathers, chunk_tile=chunk_tile,
        tile_first=tile_first, tile_last=tile_last, win=win,
    )
    return meta, percore, core_of, slot_of


# ----------------------------------------------------------------------------
# bass program
# ----------------------------------------------------------------------------

def _build_program(meta, nfeat, f1, heads, nhid, nclass):
    ntile, nlocp = meta["ntile"], meta["nlocp"]
    C, TS, TD = meta["C"], meta["TS"], meta["TD"]
    gtab_rows = CORES * nlocp
    nkf = nfeat // P          # k-chunks for layer 1 (4)
    nk2 = f1 // P             # k-chunks for layer 2 (2)
    W1C = f1 + 2 * heads      # 272
    W2C = nclass + 2          # 42
    ED1 = heads               # e_dst cols used, layer 1
    eps = 1e-20

    nc = bacc.Bacc("TRN2", num_devices=CORES, debug=False)

    # I/O
    xT = nc.dram_tensor("xT", [nkf, P, nlocp], F32, kind="ExternalInput")
    w1e = nc.dram_tensor("w1e", [nkf, P, W1C], BF16, kind="ExternalInput")
    w2e = nc.dram_tensor("w2e", [nk2, P, W2C], BF16, kind="ExternalInput")
    b1b = nc.dram_tensor("b1b", [P, f1], F32, kind="ExternalInput")
    b2b = nc.dram_tensor("b2b", [P, nclass], F32, kind="ExternalInput")
    iota = nc.dram_tensor("iota", [P, P], BF16, kind="ExternalInput")
    ident = nc.dram_tensor("ident", [P, P], BF16, kind="ExternalInput")
    srcidx = nc.dram_tensor("srcidx", [P, TS], I16, kind="ExternalInput")
    dstidx = nc.dram_tensor("dstidx", [P, TD], I16, kind="ExternalInput")
    dstlocal = nc.dram_tensor("dstlocal", [P, C], BF16, kind="ExternalInput")
    h1out = nc.dram_tensor("h1out", [nlocp, f1], F32, kind="ExternalOutput")
    logp = nc.dram_tensor("logp", [nlocp, nclass], F32, kind="ExternalOutput")

    # internal DRAM
    t1l = nc.dram_tensor("t1l", [nlocp, ROW1], BF16)
    t1g = nc.dram_tensor("t1g", [gtab_rows, ROW1], BF16, addr_space="Shared")
    ed1 = nc.dram_tensor("ed1", [nlocp, EDROW], F32)
    t2l = nc.dram_tensor("t2l", [nlocp, ROW2], BF16)
    t2g = nc.dram_tensor("t2g", [gtab_rows, ROW2], BF16, addr_space="Shared")
    ed2 = nc.dram_tensor("ed2", [nlocp, EDROW], F32)

    with tile.TileContext(nc) as tc:
        with (
            tc.tile_pool(name="const", bufs=1) as cp,
            tc.tile_pool(name="node", bufs=3) as npl,
            tc.tile_pool(name="win", bufs=3) as wp,
            tc.tile_pool(name="ep", bufs=2) as ep,
            tc.tile_pool(name="ps_scat", bufs=3, space="PSUM") as ps_scat,
            tc.tile_pool(name="ps_node", bufs=2, space="PSUM") as ps_node,
            tc.tile_pool(name="ps_l2", bufs=2, space="PSUM") as ps_l2,
        ):
            # resident constants
            w1s = cp.tile([P, nkf, W1C], BF16)
            w2s = cp.tile([P, nk2, W2C], BF16)
            b1s = cp.tile([P, f1], F32)
            b2s = cp.tile([P, nclass], F32)
            iot = cp.tile([P, P], BF16)
            idn = cp.tile([P, P], BF16)
            six = cp.tile([P, TS], I16)
            dix = cp.tile([P, TD], I16)
            dlc = cp.tile([P, C], BF16)
            nc.sync.dma_start(w1s[:], w1e.ap().rearrange("k p n -> p k n"))
            nc.sync.dma_start(w2s[:], w2e.ap().rearrange("k p n -> p k n"))
            nc.sync.dma_start(b1s[:], b1b[:, :])
            nc.sync.dma_start(b2s[:], b2b[:, :])
            nc.sync.dma_start(iot[:], iota[:, :])
            nc.sync.dma_start(idn[:], ident[:, :])
            nc.sync.dma_start(six[:], srcidx[:, :])
            nc.sync.dma_start(dix[:], dstidx[:, :])
            nc.sync.dma_start(dlc[:], dstlocal[:, :])

            # ---------------- layer 1 node phase ----------------
            for t in range(ntile):
                sl = slice(t * P, (t + 1) * P)
                xf = npl.tile([P, nkf, P], F32, tag="xf")
                nc.sync.dma_start(xf[:], xT.ap()[:, :, sl].rearrange("k p j -> p k j"))
                xb = npl.tile([P, nkf, P], BF16, tag="xb")
                nc.vector.tensor_copy(xb[:], xf[:])
                ph = ps_node.tile([P, W1C], F32, tag="pnode")
                for k in range(nkf):
                    nc.tensor.matmul(ph[:], lhsT=xb[:, k, :], rhs=w1s[:, k, :],
                                     start=(k == 0), stop=(k == nkf - 1))
                row = npl.tile([P, ROW1], BF16, tag="row1")
                nc.vector.tensor_copy(row[:, 0:f1], ph[:, 0:f1])
                nc.vector.tensor_copy(
                    row[:, f1:f1 + 2 * heads].bitcast(F32), ph[:, f1:f1 + heads])
                edt = npl.tile([P, heads], F32, tag="edrow")
                nc.vector.tensor_copy(edt[:], ph[:, f1 + heads:f1 + 2 * heads])
                nc.sync.dma_start(t1l[sl, 0:f1 + 2 * heads], row[:, 0:f1 + 2 * heads])
                nc.sync.dma_start(ed1[sl, :heads], edt[:])

            nc.gpsimd.collective_compute(
                "AllGather", mybir.AluOpType.bypass,
                replica_groups=[list(range(CORES))],
                ins=[t1l.ap().opt()], outs=[t1g.ap().opt()],
            )

            # ---------------- edge phases (shared emitter) ----------------
            def edge_phase(tabg, rowlen, edtab, edcols, emit_attention,
                           emit_epilogue, msgcols):
                open_psum = {}
                for wi, (w0, w1) in enumerate(meta["windows"]):
                    nw = w1 - w0
                    hrow = wp.tile([P, WIN, rowlen], BF16, tag="hrow")
                    for (g0, g1, b, soff) in meta["gathers"]:
                        if g0 < w0 or g0 >= w1:
                            continue
                        if _STAGE == 201:  # skip h-row gathers
                            continue
                        nidx = (g1 - g0) * P
                        b0 = b * meta["bucket_rows"]
                        bl = min(meta["bucket_rows"], gtab_rows - b0)
                        nc.gpsimd.dma_gather(
                            hrow[:, g0 - w0:g1 - w0, :],
                            tabg[b0:b0 + bl, :],
                            six[:, soff:soff + nidx // 16],
                            nidx, nidx, rowlen,
                        )
                    edr = wp.tile([P, WIN, EDROW], F32, tag="edr")
                    nidx = nw * P
                    doff = meta["dwoff"][wi]
                    if _STAGE != 202:  # 202: skip e_dst gather
                        nc.gpsimd.dma_gather(
                            edr[:, 0:nw, :], edtab[:, :],
                            dix[:, doff:doff + nidx // 16],
                            nidx, nidx, EDROW,
                        )
                    else:
                        nc.vector.memset(edr[:], 0.0)
                    if _STAGE in (20, 201, 202):  # gathers only: trivially consume
                        cons = ep.tile([P, 8], F32, tag="cons", name=f"cons{wi}")
                        nc.vector.tensor_copy(cons[:], edr[:, 0, 0:8])
                        nc.sync.dma_start(
                            h1out[wi * P:(wi + 1) * P, 0:8], cons[:])
                        continue
                    msg = emit_attention(hrow, edr, nw)
                    oh = wp.tile([P, WIN, P], BF16, tag="oh")
                    nc.vector.tensor_tensor(
                        out=oh[:, 0:nw, :],
                        in0=dlc[:, w0:w1, None].to_broadcast([P, nw, P]),
                        in1=iot[:, None, :].to_broadcast([P, nw, P]),
                        op=mybir.AluOpType.is_equal,
                    )
                    for ci in range(w0, w1):
                        t = int(meta["chunk_tile"][ci])
                        first = ci == meta["tile_first"][t]
                        last = ci == meta["tile_last"][t]
                        if first:
                            open_psum[t] = ps_scat.tile(
                                [P, msgcols], F32, tag="scat", name=f"scat{t}")
                        nc.tensor.matmul(
                            open_psum[t][:], lhsT=oh[:, ci - w0, :],
                            rhs=msg[:, ci - w0, :], start=first, stop=last)
                        if last:
                            emit_epilogue(t, open_psum.pop(t))

            # ---------------- layer 1 attention / epilogue ----------------
            def attn1(hrow, edr, nw):
                e = wp.tile([P, WIN, heads], F32, tag="e1")
                nc.vector.tensor_tensor(
                    out=e[:, 0:nw, :],
                    in0=hrow[:, 0:nw, f1:f1 + 2 * heads].bitcast(F32),
                    in1=edr[:, 0:nw, 0:heads],
                    op=mybir.AluOpType.add)
                t2 = wp.tile([P, WIN, heads], F32, tag="t1s")
                nc.vector.tensor_scalar(t2[:, 0:nw, :], e[:, 0:nw, :],
                                        SLOPE, None, mybir.AluOpType.mult)
                nc.vector.tensor_tensor(out=e[:, 0:nw, :], in0=e[:, 0:nw, :],
                                        in1=t2[:, 0:nw, :], op=mybir.AluOpType.max)
                msg = wp.tile([P, WIN, f1 + heads], BF16, tag="msg")
                exb = wp.tile([P, WIN, heads], BF16, tag="exb")
                nc.scalar.activation(exb[:, 0:nw, :], e[:, 0:nw, :],
                                     mybir.ActivationFunctionType.Exp)
                nc.scalar.activation(msg[:, 0:nw, f1:f1 + heads], e[:, 0:nw, :],
                                     mybir.ActivationFunctionType.Exp)
                nc.vector.tensor_tensor(
                    out=msg[:, 0:nw, 0:f1].rearrange("p c (h x) -> p c h x", h=heads),
                    in0=hrow[:, 0:nw, 0:f1].rearrange("p c (h x) -> p c h x", h=heads),
                    in1=exb[:, 0:nw, :, None].to_broadcast([P, nw, heads, nhid]),
                    op=mybir.AluOpType.mult)
                return msg

            def epi1(t, ps):
                sl = slice(t * P, (t + 1) * P)
                if _STAGE == 21:  # minimal epilogue: raw psum out
                    h1r = ep.tile([P, f1], F32, tag="h1f", name=f"h1r{t}")
                    nc.vector.tensor_copy(h1r[:], ps[:, 0:f1])
                    nc.sync.dma_start(h1out[sl, :], h1r[:])
                    return
                dn = ep.tile([P, heads], F32, tag="dn1")
                nc.vector.tensor_scalar(dn[:], ps[:, f1:f1 + heads], eps, None,
                                        mybir.AluOpType.add)
                rc = ep.tile([P, heads], F32, tag="rc1")
                nc.vector.reciprocal(rc[:], dn[:])
                y = ep.tile([P, f1], F32, tag="y1")
                nc.vector.tensor_tensor(
                    out=y[:].rearrange("p (h x) -> p h x", h=heads),
                    in0=ps[:, 0:f1].rearrange("p (h x) -> p h x", h=heads),
                    in1=rc[:, :, None].to_broadcast([P, heads, nhid]),
                    op=mybir.AluOpType.mult)
                nc.vector.tensor_tensor(out=y[:], in0=y[:], in1=b1s[:],
                                        op=mybir.AluOpType.add)
                # elu(y) = exp(min(y,0)) - 1 + max(y,0)
                mn = ep.tile([P, f1], F32, tag="mn1")
                nc.vector.tensor_scalar(mn[:], y[:], 0.0, None, mybir.AluOpType.min)
                em = ep.tile([P, f1], F32, tag="em1")
                nc.scalar.activation(em[:], mn[:], mybir.ActivationFunctionType.Exp)
                t3 = ep.tile([P, f1], F32, tag="t31")
                nc.vector.tensor_scalar(t3[:], y[:], 0.0, -1.0,
                                        mybir.AluOpType.max, mybir.AluOpType.add)
                h1f = ep.tile([P, f1], F32, tag="h1f")
                nc.vector.tensor_tensor(out=h1f[:], in0=em[:], in1=t3[:],
                                        op=mybir.AluOpType.add)
                nc.sync.dma_start(h1out[sl, :], h1f[:])
                if _STAGE == 22:  # skip fused L2 node compute
                    return
                h1b = ep.tile([P, f1], BF16, tag="h1b")
                nc.vector.tensor_copy(h1b[:], h1f[:])
                # fused layer-2 node compute: hh = h1 @ W2ext via PE transpose
                h1T = ep.tile([P, nk2, P], BF16, tag="h1T")
                for k in range(nk2):
                    tp = ps_l2.tile([P, P], BF16, tag="l2ps")
                    nc.tensor.transpose(tp[:], h1b[:, k * P:(k + 1) * P], idn[:])
                    nc.vector.tensor_copy(h1T[:, k, :], tp[:])
                phh = ps_l2.tile([P, W2C], F32, tag="l2ps")
                for k in range(nk2):
                    nc.tensor.matmul(phh[:], lhsT=h1T[:, k, :], rhs=w2s[:, k, :],
                                     start=(k == 0), stop=(k == nk2 - 1))
                row2 = ep.tile([P, ROW2], BF16, tag="row2")
                nc.vector.tensor_copy(row2[:, 0:nclass], phh[:, 0:nclass])
                nc.vector.tensor_copy(row2[:, nclass:nclass + 2].bitcast(F32),
                                      phh[:, nclass:nclass + 1])
                ed2t = ep.tile([P, 1], F32, tag="ed2t")
                nc.vector.tensor_copy(ed2t[:], phh[:, nclass + 1:nclass + 2])
                nc.sync.dma_start(t2l[sl, 0:nclass + 2], row2[:, 0:nclass + 2])
                nc.sync.dma_start(ed2[sl, :1], ed2t[:])

            if _STAGE >= 2:
                edge_phase(t1g, ROW1, ed1, ED1, attn1, epi1, f1 + heads)

            if _STAGE in (3, 4):
                nc.gpsimd.collective_compute(
                    "AllGather", mybir.AluOpType.bypass,
                    replica_groups=[list(range(CORES))],
                    ins=[t2l.ap().opt()], outs=[t2g.ap().opt()],
                )

            # ---------------- layer 2 attention / epilogue ----------------
            def attn2(hrow, edr, nw):
                e = wp.tile([P, WIN, 1], F32, tag="e1")
                nc.vector.tensor_tensor(
                    out=e[:, 0:nw, :],
                    in0=hrow[:, 0:nw, nclass:nclass + 2].bitcast(F32),
                    in1=edr[:, 0:nw, 0:1],
                    op=mybir.AluOpType.add)
                t2 = wp.tile([P, WIN, 1], F32, tag="t1s")
                nc.vector.tensor_scalar(t2[:, 0:nw, :], e[:, 0:nw, :],
                                        SLOPE, None, mybir.AluOpType.mult)
                nc.vector.tensor_tensor(out=e[:, 0:nw, :], in0=e[:, 0:nw, :],
                                        in1=t2[:, 0:nw, :], op=mybir.AluOpType.max)
                msg = wp.tile([P, WIN, nclass + 1], BF16, tag="msg")
                exb = wp.tile([P, WIN, 1], BF16, tag="exb")
                nc.scalar.activation(exb[:, 0:nw, :], e[:, 0:nw, :],
                                     mybir.ActivationFunctionType.Exp)
                nc.scalar.activation(msg[:, 0:nw, nclass:nclass + 1], e[:, 0:nw, :],
                                     mybir.ActivationFunctionType.Exp)
                nc.vector.tensor_tensor(
                    out=msg[:, 0:nw, 0:nclass],
                    in0=hrow[:, 0:nw, 0:nclass],
                    in1=exb[:, 0:nw, :].to_broadcast([P, nw, nclass]),
                    op=mybir.AluOpType.mult)
                return msg

            def epi2(t, ps):
                sl = slice(t * P, (t + 1) * P)
                dn = ep.tile([P, 1], F32, tag="dn1")
                nc.vector.tensor_scalar(dn[:], ps[:, nclass:nclass + 1], eps, None,
                                        mybir.AluOpType.add)
                rc = ep.tile([P, 1], F32, tag="rc1")
                nc.vector.reciprocal(rc[:], dn[:])
                y = ep.tile([P, nclass], F32, tag="y2")
                nc.vector.tensor_scalar(y[:], ps[:, 0:nclass], rc[:], None,
                                        mybir.AluOpType.mult)
                nc.vector.tensor_tensor(out=y[:], in0=y[:], in1=b2s[:],
                                        op=mybir.AluOpType.add)
                # log_softmax: z = y - ln(sum(exp(y)))  (|y| small; no max shift)
                sce = ep.tile([P, nclass], BF16, tag="sce")
                sme = ep.tile([P, 1], F32, tag="sme")
                nc.scalar.activation(sce[:], y[:], mybir.ActivationFunctionType.Exp,
                                     accum_out=sme[:])
                lse = ep.tile([P, 1], F32, tag="lse")
                nc.scalar.activation(lse[:], sme[:], mybir.ActivationFunctionType.Ln)
                z = ep.tile([P, nclass], F32, tag="z2")
                nc.vector.tensor_scalar(z[:], y[:], lse[:], None,
                                        mybir.AluOpType.subtract)
                nc.sync.dma_start(logp[sl, :], z[:])

            if _STAGE == 4:
                edge_phase(t2g, ROW2, ed2, 1, attn2, epi2, nclass + 1)

    nc.compile()
    return nc


# ----------------------------------------------------------------------------
# host orchestration
# ----------------------------------------------------------------------------

def _prepare(x, edge_index, W1, a_src1, a_dst1, b1, W2, a_src2, a_dst2, b2,
             n, nloc, ntile, bucket_rows):
    src = np.concatenate([np.asarray(edge_index[0]), np.arange(n)]).astype(np.int64)
    dst = np.concatenate([np.asarray(edge_index[1]), np.arange(n)]).astype(np.int64)
    meta, percore, core_of, slot_of = _make_schedule(
        src, dst, n, nloc, ntile, bucket_rows)
    nlocp = meta["nlocp"]

    nfeat = x.shape[1]
    f1 = W1.shape[1]
    heads, nhid = a_src1.shape
    nclass = W2.shape[1]

    W1 = np.asarray(W1, np.float32)
    W1r = W1.reshape(nfeat, heads, nhid)
    W1s = np.einsum("fhc,hc->fh", W1r, np.asarray(a_src1, np.float32))
    W1d = np.einsum("fhc,hc->fh", W1r, np.asarray(a_dst1, np.float32))
    w1e = np.concatenate([W1, W1s, W1d], 1).astype(ml_dtypes.bfloat16)
    w1e = np.ascontiguousarray(w1e.reshape(nfeat // P, P, f1 + 2 * heads))

    W2 = np.asarray(W2, np.float32)
    W2s = W2 @ np.asarray(a_src2, np.float32)[0]
    W2d = W2 @ np.asarray(a_dst2, np.float32)[0]
    w2e = np.concatenate([W2, W2s[:, None], W2d[:, None]], 1).astype(ml_dtypes.bfloat16)
    w2e = np.ascontiguousarray(w2e.reshape(f1 // P, P, nclass + 2))

    b1b = np.tile(np.asarray(b1, np.float32)[None, :], (P, 1))
    b2b = np.tile(np.asarray(b2, np.float32)[None, :], (P, 1))
    iota = np.tile(np.arange(P, dtype=np.float32)[None, :], (P, 1)).astype(
        ml_dtypes.bfloat16)
    ident = np.eye(P, dtype=np.float32).astype(ml_dtypes.bfloat16)

    x = np.asarray(x, np.float32)
    in_maps = []
    for c in range(CORES):
        nodes = np.where(core_of == c)[0]
        slots = slot_of[nodes]
        xc = np.zeros((nlocp, nfeat), np.float32)
        xc[slots] = x[nodes]
        xT = np.ascontiguousarray(xc.T.reshape(nfeat // P, P, nlocp))
        in_maps.append(dict(
            xT=xT, w1e=w1e, w2e=w2e, b1b=b1b, b2b=b2b, iota=iota, ident=ident,
            srcidx=percore[c]["srcidx"], dstidx=percore[c]["dstidx"],
            dstlocal=percore[c]["dstlocal"],
        ))
    dims = dict(nfeat=nfeat, f1=f1, heads=heads, nhid=nhid, nclass=nclass)
    return meta, in_maps, core_of, slot_of, dims


_CACHE = {}


def _install_ntff_hook():
    """The agent image's antenv lacks axon_hooks; synthesize it so
    run_bass_kernel_spmd(trace=True) can collect NTFF profiles."""
    import types

    if "antenv.axon_hooks" in sys.modules:
        return
    mod = types.ModuleType("antenv.axon_hooks")
    state = {"hook": None}
    mod.set_axon_ntff_profile_hook = lambda h: state.update(hook=h)
    mod.get_axon_ntff_profile_hook = lambda: state["hook"]
    sys.modules["antenv.axon_hooks"] = mod
    import antenv

    antenv.axon_hooks = mod
    try:
        if "/root/.axon_site" not in sys.path:
            sys.path.insert(0, "/root/.axon_site")
        from trn_agent_boot.trn_boot import _ntff_profile_via_ctypes

        mod.set_axon_ntff_profile_hook(
            _ntff_profile_via_ctypes("/opt/axon/libaxon_pjrt.so"))
    except Exception:
        pass


def _run(x, edge_index, W1, a_src1, a_dst1, b1, W2, a_src2, a_dst2, b2,
         n=N, nloc=None, bucket_rows=BUCKET, trace=False):
    nloc = nloc if nloc is not None else (n + CORES - 1) // CORES
    ntile = (nloc + P - 1) // P
    meta, in_maps, core_of, slot_of, dims = _prepare(
        x, edge_index, W1, a_src1, a_dst1, b1, W2, a_src2, a_dst2, b2,
        n, nloc, ntile, bucket_rows)

    key = (meta["C"], meta["TS"], meta["TD"], ntile, n,
           hash(tuple(meta["gathers"])), hash(meta["chunk_tile"].tobytes()))
    if key not in _CACHE:
        _CACHE[key] = _build_program(meta, dims["nfeat"], dims["f1"],
                                     dims["heads"], dims["nhid"], dims["nclass"])
    nc = _CACHE[key]

    if trace:
        _install_ntff_hook()
    res = run_bass_kernel_spmd(nc, in_maps, list(range(CORES)), trace=trace)
    f1, nclass = dims["f1"], dims["nclass"]
    h1 = np.empty((n, f1), np.float32)
    lp = np.empty((n, nclass), np.float32)
    for c in range(CORES):
        nodes = np.where(core_of == c)[0]
        slots = slot_of[nodes]
        h1[nodes] = res.results[c]["h1out"][slots]
        lp[nodes] = res.results[c]["logp"][slots]
    return (lp, h1), res.exec_time_ns


def kernel(x, edge_index, W1, a_src1, a_dst1, b1, W2, a_src2, a_dst2, b2):
    (lp, h1), _ = _run(x, edge_index, W1, a_src1, a_dst1, b1,
                       W2, a_src2, a_dst2, b2)
    return lp, h1
